# revision 14
# baseline (speedup 1.0000x reference)
"""Trainium2 Bass kernel for nn_Attention_46454366273781 (sparse_attention).

Reference computation (T=2048, B=32, N=1024, H=8, K=128, K2=16):
    X = einsum('tbn,hkn->bthk', hyp, Wmh) + bmh          # per-head projections
    m = X.mean(axis=1)                                   # mean over time
    g = tanh(X @ W.T + bW) * tanh(m @ Wm.T + bWm)[:,None]
    s = g @ Wh + bWh ; a = softmax(s, axis=time)
    c = einsum('bth,bthk->bhk', a, X) ; out = c.reshape(B, H*K)

Key algebra: X itself is never needed on device.
  * scoring:  X @ W.T + bW  =  hyp @ WS.T + bSp   with WS = W @ Wmh (per head)
  * gate:     m @ Wm.T + bWm = mean_t(hyp) @ WSm.T + bSm,  WSm = Wm @ Wmh
  * gate fold: s = Wh^T (g1 * mw) = (Wh*mw)^T g1  -> fold mw into Wh per batch
  * output:   softmax weights sum to 1, so
              c_bh = (a_bh^T hyp_b) @ Wmh_h^T + bmh_h  (the v-trick)

DMA-roofline design: hyp ships in fp8 (e3m4) in BOTH layouts (N-major for
scoring, T-major for mean/weighted-sum).  Weights with small magnitudes
(WS, WSm, Wmh) are pre-scaled by 256 on host to stay in e3m4's normal
range; the 1/256 factors fold into activation scales / the diag matmul.
All contractions accumulate in f32 PSUM.

PE work is oriented so matmul outputs have tiny free dims:
  * mean:  z^T per n-chunk = hN_chunk^T @ ones         (out 128x1)
  * v:     v^T per n-chunk = hN_chunk^T @ aT           (out 128x8)
  * aT:    s_exp_chunk^T @ diag(sinv/256)              (out 128x8)
Scoring is the only large-free matmul (out 128x512 per chunk).

The emission order software-pipelines batches: batch b's serial
gate->softmax->v chain hides under batch b+1's scoring (engine queues are
in-order).  The c-phase for batches 0..2 runs inside the last batch's exp
wait; wmhT streams after all hyp so the last batch's data arrives sooner.

Sharding: data-parallel over batch B across 8 cores (4 batches/core).
bWh cancels inside the softmax.
"""

import numpy as np
import ml_dtypes

T, B, N, H = 2048, 32, 1024, 8
K, K2 = 128, 16          # per-head dim, attention hidden per head
NCORES = 8
BL = B // NCORES         # batches per core
TC = 512                 # time chunk for scoring matmul free dim
TCH = T // TC            # time chunks (scoring)
NCH = N // 128           # contraction chunks over N
T128 = T // 128          # 128-sized time chunks
NQ = 4                   # hypN DMA quarters per batch
QT = T128 // NQ          # t-chunks per quarter
SCL = 256.0              # fp8 weight pre-scale (power of 2)

_cache = {}


def _build_nc():
    import concourse.mybir as mybir
    import concourse.tile as tile
    from concourse import bacc

    bf16 = mybir.dt.bfloat16
    f8 = mybir.dt.float8e3
    f8e4 = mybir.dt.float8e4
    DR = mybir.MatmulPerfMode.DoubleRow
    f32 = mybir.dt.float32
    AF = mybir.ActivationFunctionType
    AX = mybir.AxisListType
    OP = mybir.AluOpType

    nc = bacc.Bacc("TRN2")
    hypT_d = nc.dram_tensor("hypT", (BL, NQ, 128, NCH, TC), f8e4, kind="ExternalInput")
    hypN_d = nc.dram_tensor("hypN", (BL, 128, T128, N), f8, kind="ExternalInput")
    WST_d = nc.dram_tensor("WST", (128, NCH, 128), f8e4, kind="ExternalInput")
    WSmT_d = nc.dram_tensor("WSmT", (128, NCH, 128), f8, kind="ExternalInput")
    whD_d = nc.dram_tensor("whD", (K, H), bf16, kind="ExternalInput")
    bSp_d = nc.dram_tensor("bSp", (128, 1), f32, kind="ExternalInput")
    bSm_d = nc.dram_tensor("bSm", (128, 1), f32, kind="ExternalInput")
    bmhL_d = nc.dram_tensor("bmhL", (1, H, K), bf16, kind="ExternalInput")
    wmhT_d = nc.dram_tensor("wmhT", (128, H, NCH, K), f8, kind="ExternalInput")
    ones_d = nc.dram_tensor("ones1", (128, 1), bf16, kind="ExternalInput")
    id8_d = nc.dram_tensor("id8", (8, 8), bf16, kind="ExternalInput")
    out_d = nc.dram_tensor("out", (128, BL, H), f32, kind="ExternalOutput")

    with tile.TileContext(nc) as tc, \
         tc.tile_pool(name="wpool", bufs=1) as wpool, \
         tc.tile_pool(name="hTp", bufs=BL * NQ) as hTp, \
         tc.tile_pool(name="hNp", bufs=2 * NQ + 1) as hNp, \
         tc.tile_pool(name="gp", bufs=2) as gp, \
         tc.tile_pool(name="smallp", bufs=2) as smallp, \
         tc.tile_pool(name="psA", bufs=2, space="PSUM") as psA, \
         tc.tile_pool(name="psBig", bufs=1, space="PSUM") as psBig, \
         tc.tile_pool(name="psSm", bufs=2, space="PSUM") as psSm:

        # ---- act-table warmup: pull the one-time table load off the
        # critical path (Tanh/Exp/Copy all live in one table set)
        scr = wpool.tile([128, 1], bf16)
        nc.vector.memset(scr, 0.0)
        scr2 = wpool.tile([128, 1], bf16)
        nc.scalar.activation(out=scr2, in_=scr, func=AF.Tanh)

        # hT quarters on SP, hN quarters on Pool: the two queues alternate
        # 1:1 at the shared DMA device, so batch p's hT and hN finish
        # together every ~11.7us -- and the compile-time scheduler (which
        # charges transfers to the issuing engine) sees the same overlap.
        hT = [[hTp.tile([128, NCH, TC], f8e4, tag="hT", name=f"hT_{bl}_{q}")
               for q in range(NQ)] for bl in range(BL)]
        hNq = [[hNp.tile([128, QT, N], f8, tag="hN", name=f"hN_{bl}_{q}")
                for q in range(NQ)] for bl in range(BL)]
        for bl in range(BL):
            for q in range(NQ):
                nc.sync.dma_start(out=hT[bl][q], in_=hypT_d[bl, q])
                nc.gpsimd.dma_start(out=hNq[bl][q],
                                    in_=hypN_d[bl, :, q * QT:(q + 1) * QT, :])
        wmhT = wpool.tile([128, H, NCH, K], f8)
        nc.sync.dma_start(out=wmhT, in_=wmhT_d[:])

        # ---- small weights on the Act queue: early, off the big streams ----
        WST = wpool.tile([128, NCH, 128], f8e4)
        nc.scalar.dma_start(out=WST, in_=WST_d[:])
        bSp = wpool.tile([128, 1], f32)
        nc.scalar.dma_start(out=bSp, in_=bSp_d[:])
        ones1 = wpool.tile([128, 1], bf16)
        nc.scalar.dma_start(out=ones1, in_=ones_d[:])
        WSmT = wpool.tile([128, NCH, 128], f8)
        nc.scalar.dma_start(out=WSmT, in_=WSmT_d[:])
        whD = wpool.tile([K, H], bf16)
        nc.scalar.dma_start(out=whD, in_=whD_d[:])
        bSm = wpool.tile([128, 1], f32)
        nc.scalar.dma_start(out=bSm, in_=bSm_d[:])
        id8 = wpool.tile([8, 8], bf16)
        nc.scalar.dma_start(out=id8, in_=id8_d[:])
        bmhL = wpool.tile([1, H, K], bf16)
        nc.scalar.dma_start(out=bmhL, in_=bmhL_d[:])
        out_sb = wpool.tile([128, BL, H], f32)

        # ---- per-batch state ----
        g1s, s_exps, diag8s, aTs, vTs, whDms, small_ps = ({} for _ in range(7))
        A0 = NCH + 1                 # aT region offset in the small psum tile
        V0 = NCH + 1 + T128 * 8      # vT region offset

        def emit_A(bl, tci):
            # scoring chunk: ps = WS256 @ hyp^T ; g1 = tanh(ps/256 + bSp)
            if tci == 0:
                g1s[bl] = gp.tile([128, T], bf16, tag="g1", name=f"g1_{bl}")
            ps = psA.tile([128, TC], f32, tag="psA", name=f"ps_{bl}_{tci}")
            for n2 in range(NCH // 2):
                nc.tensor.matmul(ps, lhsT=WST[:, 2 * n2:2 * n2 + 2, :],
                                 rhs=hT[bl][tci][:, 2 * n2:2 * n2 + 2, :],
                                 start=(n2 == 0), stop=(n2 == NCH // 2 - 1),
                                 perf_mode=DR)
            nc.scalar.activation(out=g1s[bl][:, tci * TC:(tci + 1) * TC],
                                 in_=ps, func=AF.Tanh,
                                 bias=bSp, scale=1.0 / SCL)

        def emit_z(bl):
            # mean pass: z^T columns via ones-matmuls on hN chunks
            sp = psSm.tile([128, NCH + 1 + T128 * 8 + NCH * 8], f32,
                           tag="ps_sm", name=f"sp_{bl}")
            small_ps[bl] = sp
            for q in range(NQ):
                for t2 in range(QT):
                    for n in range(NCH):
                        nc.tensor.matmul(
                            sp[:, n:n + 1],
                            lhsT=hNq[bl][q][:, t2, n * 128:(n + 1) * 128],
                            rhs=ones1,
                            start=(q == 0 and t2 == 0),
                            stop=(q == NQ - 1 and t2 == QT - 1),
                            skip_group_check=True)

        def emit_zmw(bl):
            # gate: mw = tanh(WSm256 @ z / (256*T) + bSm); fold into Wh
            sp = small_ps[bl]
            zbf = smallp.tile([128, NCH], bf16, tag="zbf", name=f"zbf_{bl}")
            nc.vector.tensor_copy(out=zbf, in_=sp[:, 0:NCH])
            for n in range(NCH):
                nc.tensor.matmul(sp[:, NCH:NCH + 1], lhsT=WSmT[:, n, :],
                                 rhs=zbf[:, n:n + 1],
                                 start=(n == 0), stop=(n == NCH - 1),
                                 skip_group_check=True)
            mwP = smallp.tile([128, 1], f32, tag="mwP", name=f"mwP_{bl}")
            nc.scalar.activation(out=mwP, in_=sp[:, NCH:NCH + 1], func=AF.Tanh,
                                 bias=bSm, scale=1.0 / (SCL * T))
            whDm = smallp.tile([K, H], bf16, tag="whDm", name=f"whDm_{bl}")
            nc.scalar.activation(out=whDm, in_=whD, func=AF.Copy, scale=mwP)
            whDms[bl] = whDm

        def emit_s(bl):
            # scores s = whDm^T g1 ; one big exp with fused softmax-sum
            s_exps[bl] = gp.tile([8, T], bf16, tag="s_exp", name=f"s_exp_{bl}")
            ps_s = psBig.tile([8, T], f32, tag="psBig", name=f"ps_s_{bl}")
            for tci in range(TCH):
                tsl = slice(tci * TC, (tci + 1) * TC)
                nc.tensor.matmul(ps_s[:, tsl], lhsT=whDms[bl],
                                 rhs=g1s[bl][:, tsl], start=True, stop=True,
                                 skip_group_check=True)
            ssum = smallp.tile([8, 1], f32, tag="ssum", name=f"ssum_{bl}")
            nc.scalar.activation(out=s_exps[bl], in_=ps_s, func=AF.Exp,
                                 accum_out=ssum)
            sinv = smallp.tile([8, 1], f32, tag="sinv", name=f"sinv_{bl}")
            nc.vector.reciprocal(sinv, ssum)
            diag8 = smallp.tile([8, 8], bf16, tag="diag8", name=f"diag8_{bl}")
            nc.vector.tensor_scalar_mul(diag8, id8, sinv)
            diag8s[bl] = diag8

        def emit_aT(bl):
            # aT chunks = s_exp_chunk^T @ diag(sinv/256)
            sp = small_ps[bl]
            for tcc in range(T128):
                nc.tensor.matmul(sp[:, A0 + tcc * 8:A0 + (tcc + 1) * 8],
                                 lhsT=s_exps[bl][:, tcc * 128:(tcc + 1) * 128],
                                 rhs=diag8s[bl], start=True, stop=True,
                                 skip_group_check=True)
            aT = smallp.tile([128, T128 * 8], bf16, tag="aT", name=f"aT_{bl}")
            nc.vector.tensor_copy(out=aT, in_=sp[:, A0:A0 + T128 * 8])
            aTs[bl] = aT

        def emit_v(bl):
            # v^T per n-chunk = hN_chunk^T @ aT  (out free = 8)
            sp = small_ps[bl]
            for n in range(NCH):
                for tcc in range(T128):
                    nc.tensor.matmul(
                        sp[:, V0 + n * 8:V0 + (n + 1) * 8],
                        lhsT=hNq[bl][tcc // QT][:, tcc % QT,
                                                n * 128:(n + 1) * 128],
                        rhs=aTs[bl][:, tcc * 8:(tcc + 1) * 8],
                        start=(tcc == 0), stop=(tcc == T128 - 1),
                        skip_group_check=True)
            vT = smallp.tile([128, NCH, 8], bf16, tag="vT", bufs=BL,
                             name=f"vT_{bl}")
            nc.vector.tensor_copy(out=vT, in_=sp[:, V0:V0 + NCH * 8]
                                  .rearrange("p (n h) -> p n h", n=NCH))
            vTs[bl] = vT

        ps_c = [None]

        def emit_c(bl):
            # c^T = Wmh256 @ v/256 + bmh (bias folded in as a rank-1 term)
            if ps_c[0] is None:
                ps_c[0] = psSm.tile([128, BL, H], f32, tag="ps_sm",
                                    name="ps_c")
            for h in range(H):
                for n in range(NCH):
                    nc.tensor.matmul(ps_c[0][:, bl, h:h + 1],
                                     lhsT=wmhT[:, h, n, :],
                                     rhs=vTs[bl][:, n, h:h + 1],
                                     start=(n == 0), stop=False,
                                     skip_group_check=True)
                nc.tensor.matmul(ps_c[0][:, bl, h:h + 1],
                                 lhsT=bmhL[:, h, :],
                                 rhs=ones1[0:1, :],
                                 start=False, stop=True,
                                 skip_group_check=True)

        def emit_s_chunk(bl, tci, ssum_parts):
            if tci == 0:
                s_exps[bl] = gp.tile([8, T], bf16, tag="s_exp",
                                     name=f"s_exp_{bl}")
                small_ps[f"s{bl}"] = psBig.tile([8, T], f32, tag="psBig",
                                                name=f"ps_s_{bl}")
            ps_s = small_ps[f"s{bl}"]
            tsl = slice(tci * TC, (tci + 1) * TC)
            nc.tensor.matmul(ps_s[:, tsl], lhsT=whDms[bl],
                             rhs=g1s[bl][:, tsl], start=True, stop=True,
                             skip_group_check=True)
            nc.scalar.activation(out=s_exps[bl][:, tsl], in_=ps_s[:, tsl],
                                 func=AF.Exp,
                                 accum_out=ssum_parts[:, tci:tci + 1])

        # ---- software-pipelined emission ----
        for tci in range(TCH):
            emit_A(0, tci)
        emit_z(0)
        emit_zmw(0)

        last = BL - 1
        for p in range(1, BL):
            emit_s(p - 1)
            emit_A(p, 0)
            emit_A(p, 1)
            emit_A(p, 2)
            emit_aT(p - 1)
            if p != last:
                emit_A(p, 3)
            emit_v(p - 1)
            emit_z(p)
            emit_zmw(p)

        # last batch: s+exp chase the scoring quarters; the gate is ready
        # early because hN finishes before hT.  g1(q3) must come after
        # whDm on the in-order Act queue, so A(3,3) is emitted here.
        ssum_parts = smallp.tile([8, TCH], f32, tag="ssum_parts",
                                 name="ssum_parts")
        emit_s_chunk(last, 0, ssum_parts)
        emit_s_chunk(last, 1, ssum_parts)
        emit_s_chunk(last, 2, ssum_parts)
        emit_A(last, 3)
        emit_s_chunk(last, 3, ssum_parts)
        ssum = smallp.tile([8, 1], f32, tag="ssum", name="ssum_l")
        nc.vector.reduce_sum(out=ssum, in_=ssum_parts, axis=AX.X)
        sinv = smallp.tile([8, 1], f32, tag="sinv", name="sinv_l")
        nc.vector.reciprocal(sinv, ssum)
        diag8l = smallp.tile([8, 8], bf16, tag="diag8", name="diag8_l")
        nc.vector.tensor_scalar_mul(diag8l, id8, sinv)
        diag8s[last] = diag8l
        for bl in range(last):
            emit_c(bl)
        emit_aT(last)
        emit_v(last)
        emit_c(last)
        nc.vector.tensor_copy(out=out_sb, in_=ps_c[0])

        nc.sync.dma_start(out=out_d[:], in_=out_sb)

    nc.compile()
    return nc


def _prep_inputs(hyp, Wmh, bmh, W, bW, Wm, bWm, Wh, bWh):
    """Host-side sharding + layout prep (numpy only)."""
    bf = ml_dtypes.bfloat16
    f8 = ml_dtypes.float8_e3m4
    f8e4 = ml_dtypes.float8_e4m3
    hyp = np.asarray(hyp, np.float32)
    Wmh = np.asarray(Wmh, np.float32)
    bmh = np.asarray(bmh, np.float32)
    W = np.asarray(W, np.float32)
    bW = np.asarray(bW, np.float32)
    Wm = np.asarray(Wm, np.float32)
    bWm = np.asarray(bWm, np.float32)
    Wh = np.asarray(Wh, np.float32)

    # quantize hyp once, then shuffle bytes for the two layouts
    hyp8 = hyp.astype(f8)                                       # (T, B, N)
    # hypT[b, tq, p, n, t'] = hyp[512*tq + t', b, 128n+p]
    hypT_all = np.ascontiguousarray(
        hyp.astype(f8e4).transpose(1, 2, 0).reshape(B, NCH, 128, NQ, TC)
        .transpose(0, 3, 2, 1, 4))
    # hypN[b, p, tc, n] = hyp[128*tc+p, b, n]
    hypN_all = np.ascontiguousarray(
        hyp8.transpose(1, 0, 2).reshape(B, T128, 128, N).transpose(0, 2, 1, 3))

    # fused scoring weights: WS[h*16+q, n] = sum_k W[q,k] Wmh[h,k,n]
    WS = np.einsum('qk,hkn->hqn', W, Wmh).reshape(128, N) * SCL
    WST = np.ascontiguousarray(
        WS.T.reshape(NCH, 128, 128).transpose(1, 0, 2)).astype(f8e4)
    bSp = (np.einsum('qk,hk->hq', W, bmh).reshape(128)
           + np.tile(bW, H)).astype(np.float32).reshape(128, 1)

    WSm = np.einsum('qk,hkn->hqn', Wm, Wmh).reshape(128, N) * SCL
    WSmT = np.ascontiguousarray(
        WSm.T.reshape(NCH, 128, 128).transpose(1, 0, 2)).astype(f8)
    bSm = (np.einsum('qk,hk->hq', Wm, bmh).reshape(128)
           + np.tile(bWm, H)).astype(np.float32).reshape(128, 1)

    whD = np.zeros((K, H), dtype=np.float32)
    for h in range(H):
        whD[h * K2:(h + 1) * K2, h] = Wh
    whD = whD.astype(bf)
    bmhL = np.ascontiguousarray(bmh.reshape(1, H, K)).astype(bf)

    # wmhT[p, h, n, k] = Wmh[h, k, 128n+p] * 256
    wmhT = np.ascontiguousarray(
        (Wmh * SCL).transpose(2, 0, 1).reshape(NCH, 128, H, K)
        .transpose(1, 2, 0, 3)).astype(f8)

    ones1 = np.ones((128, 1), dtype=bf)
    id8 = (np.eye(8, dtype=np.float32) / SCL).astype(bf)

    in_maps = []
    for c in range(NCORES):
        sl = slice(c * BL, (c + 1) * BL)
        in_maps.append({
            "hypT": np.ascontiguousarray(hypT_all[sl]),
            "hypN": np.ascontiguousarray(hypN_all[sl]),
            "WST": WST, "WSmT": WSmT, "whD": whD,
            "bSp": bSp, "bSm": bSm, "bmhL": bmhL, "wmhT": wmhT,
            "ones1": ones1, "id8": id8,
        })
    return in_maps


def kernel(hyp, Wmh, bmh, W, bW, Wm, bWm, Wh, bWh,
           dan_hidden_size=None, attention_hidden_size=None,
           multihead_size=None, **_):
    from concourse.bass_utils import run_bass_kernel_spmd

    in_maps = _prep_inputs(hyp, Wmh, bmh, W, bW, Wm, bWm, Wh, bWh)
    if "nc" not in _cache:
        _cache["nc"] = _build_nc()
    res = run_bass_kernel_spmd(_cache["nc"], in_maps, core_ids=list(range(NCORES)))
    # out is (128=k, BL, H) per core -> (BL, H, K) -> (BL, N)
    out = np.concatenate([r["out"].transpose(1, 2, 0).reshape(BL, N)
                          for r in res.results], axis=0)
    return out.astype(np.float32)
